# revision 3
# baseline (speedup 1.0000x reference)
"""Sigmoid-attention kernel for Trainium2, SPMD over 8 NeuronCores.

Reference computation (per batch b, head h):
    q = (x @ Wq_h) * SCALE ; k = x @ Wk_h ; v = x[:, :, h*64:(h+1)*64]
    out_h = sigmoid((q + bias_h) @ k^T) @ v
Sharding: 8 cores = 4 batches x 2 head-groups (4 heads each).
Each core computes its 4 heads independently; no collectives.

Per-core kernel layout (all matmuls contract along SBUF partitions):
    xT   [512, 2048]  bf16  -- x[b] transposed on host (features on partitions)
    proj: qbT_h = (Wq_h^T @ xT) * SCALE + bias_h   [64, 2048] per head
          kT_h  =  Wk_h^T @ xT                     [64, 2048]
    scores: S^T(j) = kT_h(j-tile)^T-contract-> [j 128, i 512] chunks, sigmoid -> P^T bf16
    out:  O[i 64part?]: O^T... O = P^T-contract-> out^T [64, 2048] per head (e on partitions)
Host re-transposes out^T into the reference layout.
"""
import sys

import numpy as np
import ml_dtypes

try:
    import concourse.bass as bass  # noqa: F401
except ImportError:
    sys.path.insert(0, "/opt/trn_rl_repo")
import concourse.tile as tile
from concourse import bacc, mybir
from concourse.bass_utils import run_bass_kernel_spmd

BF16 = mybir.dt.bfloat16
F32 = mybir.dt.float32
bf16 = ml_dtypes.bfloat16

B, N, DIM = 4, 2048, 512
HEADS, DK = 8, 64
SCALE = DK ** -0.5
NCORES = 8
HPG = 4            # heads per group (= per core)
GD = HPG * DK      # 256: group feature width
DC = DIM // 128    # 4 d-chunks (contraction tiles for projections)
NIC = N // 512     # 4 i-chunks
NJ = N // 128      # 16 j-tiles
JG = 2             # j-tiles per sigmoid group ([128, 1024] ACT instructions)

ACT = mybir.ActivationFunctionType


def _build():
    nc = bacc.Bacc("TRN2", target_bir_lowering=False, debug=False)
    xT = nc.declare_dram_parameter("xT", [DIM, N], BF16, isOutput=False)
    wq = nc.declare_dram_parameter("wq", [DIM, GD], BF16, isOutput=False)
    wk = nc.declare_dram_parameter("wk", [DIM, GD], BF16, isOutput=False)
    vv = nc.declare_dram_parameter("v", [N, GD], BF16, isOutput=False)
    bias = nc.declare_dram_parameter("bias", [GD, 1], F32, isOutput=False)
    out = nc.declare_dram_parameter("out", [HPG, DK, N], F32, isOutput=True)

    with tile.TileContext(nc) as tc:
        with (
            tc.tile_pool(name="const", bufs=1) as cpool,
            tc.tile_pool(name="qk", bufs=2) as qkpool,
            tc.tile_pool(name="pp", bufs=3) as ppool,
            tc.tile_pool(name="osb", bufs=2) as opool,
            tc.tile_pool(name="ps_proj", bufs=1, space="PSUM") as pjpool,
            tc.tile_pool(name="ps_s", bufs=2, space="PSUM") as spool,
            tc.tile_pool(name="ps_o", bufs=2, space="PSUM") as oppool,
        ):
            # ---- constants ----
            xt_t = []
            for dc in range(DC):
                t = cpool.tile([128, N], BF16, name=f"xt{dc}")
                nc.sync.dma_start(t[:], xT[dc * 128:(dc + 1) * 128, :])
                xt_t.append(t)
            wq_t, wk_t = [], []
            for dc in range(DC):
                t = cpool.tile([128, GD], BF16, name=f"wqt{dc}")
                nc.sync.dma_start(t[:], wq[dc * 128:(dc + 1) * 128, :])
                wq_t.append(t)
                t = cpool.tile([128, GD], BF16, name=f"wkt{dc}")
                nc.sync.dma_start(t[:], wk[dc * 128:(dc + 1) * 128, :])
                wk_t.append(t)
            # v rearranged so partition p holds v[jc*128+p, :] for each j-chunk jc
            v_t = cpool.tile([128, NJ * GD], BF16, name="vt")
            nc.sync.dma_start(
                v_t[:].rearrange("p (jc e) -> p jc e", jc=NJ),
                vv.rearrange("(jc p) e -> p jc e", p=128),
            )
            bias_t = []
            for h in range(HPG):
                t = cpool.tile([64, 1], F32, name=f"bias{h}")
                nc.sync.dma_start(t[:], bias[h * 64:(h + 1) * 64, :])
                bias_t.append(t)

            for h in range(HPG):
                # ---- projections: qbT/kT [64, N] ----
                qbT = qkpool.tile([64, N], BF16, tag="qbT", name=f"qbT{h}")
                kT = qkpool.tile([64, N], BF16, tag="kT", name=f"kT{h}")
                for ic in range(NIC):
                    pq = pjpool.tile([64, 512], F32, tag="pq", name=f"pq{h}_{ic}")
                    pk = pjpool.tile([64, 512], F32, tag="pk", name=f"pk{h}_{ic}")
                    cs = slice(ic * 512, (ic + 1) * 512)
                    for dc in range(DC):
                        nc.tensor.matmul(
                            pq[:], wq_t[dc][:, h * DK:(h + 1) * DK], xt_t[dc][:, cs],
                            start=(dc == 0), stop=(dc == DC - 1),
                        )
                    for dc in range(DC):
                        nc.tensor.matmul(
                            pk[:], wk_t[dc][:, h * DK:(h + 1) * DK], xt_t[dc][:, cs],
                            start=(dc == 0), stop=(dc == DC - 1),
                        )
                    nc.scalar.activation(qbT[:, cs], pq[:], ACT.Identity,
                                         bias=bias_t[h][:, :], scale=float(SCALE))
                    nc.vector.tensor_copy(kT[:, cs], pk[:])

                # ---- attention ----
                out_sb = opool.tile([DK, N], F32, tag="osb", name=f"osb{h}")
                for ic in range(NIC):
                    ics = slice(ic * 512, (ic + 1) * 512)
                    o_ps = oppool.tile([DK, 512], F32, tag="ops", name=f"ops{h}_{ic}")
                    for jg in range(NJ // JG):
                        s_ps = spool.tile([128, JG * 512], F32, tag="sg",
                                          name=f"s{h}_{ic}_{jg}")
                        for q in range(JG):
                            j = jg * JG + q
                            nc.tensor.matmul(
                                s_ps[:, q * 512:(q + 1) * 512],
                                kT[:, j * 128:(j + 1) * 128],
                                qbT[:, ics],
                                start=True, stop=True,
                            )
                        p_sb = ppool.tile([128, JG * 512], BF16, tag="pg",
                                          name=f"p{h}_{ic}_{jg}")
                        nc.scalar.activation(p_sb[:], s_ps[:], ACT.Sigmoid)
                        for q in range(JG):
                            j = jg * JG + q
                            nc.tensor.matmul(
                                o_ps[:],
                                v_t[:, j * GD + h * DK: j * GD + (h + 1) * DK],
                                p_sb[:, q * 512:(q + 1) * 512],
                                start=(j == 0), stop=(j == NJ - 1),
                            )
                    nc.vector.tensor_copy(out_sb[:, ics], o_ps[:])
                nc.sync.dma_start(out[h], out_sb[:])
    nc.compile()
    return nc


_NC_CACHE = None


def _get_nc():
    global _NC_CACHE
    if _NC_CACHE is None:
        _NC_CACHE = _build()
    return _NC_CACHE


def kernel(x, Wq, Wk, rel_content_bias):
    x = np.asarray(x, dtype=np.float32)
    Wq = np.asarray(Wq, dtype=np.float32)
    Wk = np.asarray(Wk, dtype=np.float32)
    rb = np.asarray(rel_content_bias, dtype=np.float32)

    nc = _get_nc()

    xT_b = [np.ascontiguousarray(x[b].T).astype(bf16) for b in range(B)]
    wq_bf = Wq.astype(bf16)
    wk_bf = Wk.astype(bf16)
    bias_flat = rb.reshape(HEADS * DK, 1)  # [512, 1] head-major

    in_maps = []
    for c in range(NCORES):
        b, g = divmod(c, 2)
        gs = slice(g * GD, (g + 1) * GD)
        in_maps.append({
            "xT": xT_b[b],
            "wq": np.ascontiguousarray(wq_bf[:, gs]),
            "wk": np.ascontiguousarray(wk_bf[:, gs]),
            "v": np.ascontiguousarray(x[b, :, gs]).astype(bf16),
            "bias": np.ascontiguousarray(bias_flat[g * GD:(g + 1) * GD]),
        })

    res = run_bass_kernel_spmd(nc, in_maps, core_ids=list(range(NCORES)))

    out_full = np.empty((B, N, DIM), dtype=np.float32)
    for c in range(NCORES):
        b, g = divmod(c, 2)
        oc = res.results[c]["out"]  # [HPG, DK, N]
        for h in range(HPG):
            col = g * GD + h * DK
            out_full[b, :, col:col + DK] = oc[h].T
    return out_full
